# revision 27
# baseline (speedup 1.0000x reference)
"""Trainium2 Bass kernel for nn_EncoderGRU (B=128, T=512, D=64, H=512).

Strategy
--------
Pure data-parallel over batch: 8 cores x 16 batch rows each. The T=512 GRU
recurrence is inherently sequential, so per-step critical-path latency is
everything.

Per step the PE computes gate pre-activations with h as the stationary
operand and weights moving, with 4-way column tiling (strip q = PSUM
partitions 32q..32q+15 carries the 512-wide gate block for H-chunk q).
Strip layout is [r | hn | zb | in] so the matmul splits into subpass A
(cols 0:256 = r,hn) and subpass B (cols 256:512): sigmoid(r) fires as soon
as A's four k-groups finish, overlapping with B's streaming.

Gate math uses zbar = sigmoid(-z_pre) (z negated host-side) and
z = 1 - zbar, so

    h' = zbar*n + z*h  =  m1 + m2

and the final add is executed ON THE PE as two accumulating bf16
transposes T(m2)+T(m1) into PSUM - fusing the h'-sum with the
batch->H-partition transpose the next step's stationary needs. The
batch-layout copy h_bl' = m1+m2 runs on GpSimd off the critical path.

Critical chain per step:
    A-passes -> sigmoid(r) -> r*hn -> +in -> tanh -> zbar*n -> T(m1) -> cast

All h-independent matmuls (bias/delta K4 row, teacher-forcing x@W_ih, the
interleaved output projection) run on the PE during the gate phase, keeping
the PE HAM-warm.

samp_mask is read host-side; the kernel is specialized per step at build
time (teacher forcing vs autoregressive; AR folds the prev_out feedback
into the recurrent weights: inp = [h @ W_out + b_out, delta] =>
gi = h @ (W_out @ Wih_d.T) + ...).
"""

import os
import sys
import numpy as np

sys.path.insert(0, "/opt/trn_rl_repo")

DEBUG_DUMP = bool(os.environ.get("KERNEL_DEBUG_DUMP"))

B, T, D, H = 128, 512, 64, 512
NC = 8            # cores
BL = B // NC      # local batch = 16
OUT_BLK = 32      # steps per output-projection block
RING = 64         # h-history ring slots (must be multiple of OUT_BLK)

_CACHE = {}


def _ap(base, offset_add, ap_dims):
    """Clone an AP keeping the partition dim, replacing the free dims (SBUF)."""
    import concourse.bass as bass
    return bass.AP(
        tensor=base.tensor,
        offset=base.offset + offset_add,
        ap=[base.ap[0]] + ap_dims,
    )


def _apd(base, offset_add, ap_dims):
    """Clone a DRAM AP with fully custom dims."""
    import concourse.bass as bass
    return bass.AP(
        tensor=base.tensor,
        offset=base.offset + offset_add,
        ap=ap_dims,
    )


def _patch_drain_wait_limit():
    """The walrus build in this image rejects >1 sync wait on a CTRL/Drain
    instruction ("Too many sync wait commands"). Spread the kernel-tail
    drain's waits across multiple drain instructions."""
    import bass_rust
    import concourse.tile as tile
    from concourse.vector_clock import ScopedClock

    if getattr(tile.TileContext, "_drain_patched", False):
        return
    MAXW = 1

    def _drain_and_barrier(self, tick_clock, wait_clock):
        drain_inst = self.nc.sync.drain()
        wait_clock.add_sem_waits(
            drain_inst.ins, ScopedClock({None: tick_clock.global_clock})
        )
        si = drain_inst.ins.sync_info
        waits = list(si.on_wait or [])
        if len(waits) > MAXW:
            si.on_wait = waits[:MAXW]
            rest = waits[MAXW:]
            for i in range(0, len(rest), MAXW):
                d2 = self.nc.sync.drain()
                chunk = rest[i:i + MAXW]
                si2 = d2.ins.sync_info
                if si2 is None:
                    d2.ins.sync_info = bass_rust.SyncInfo(
                        on_wait=chunk, on_update=[])
                else:
                    si2.on_wait = chunk
        self.nc.all_engine_barrier()
        assert self.sems is not None
        popped = self.nc._tile_sem_poison_stack.pop()
        assert popped is self._sem_poison
        self.nc.clear_and_free_semaphores(list(self.sems.allocated().values()))
        self.nc.all_engine_barrier()

    tile.TileContext._drain_and_barrier = _drain_and_barrier
    tile.TileContext._drain_patched = True


def _split_multi_waits(nc):
    """The walrus build in this image accepts at most ONE sync wait per
    instruction. Hoist extra waits onto preceding single-wait NoOps on the
    same engine (engine sequencers execute in order, so semantics hold)."""
    import bass_rust
    from concourse import mybir

    n_new = 0
    for fn in nc.m.functions:
        for bb in fn.blocks:
            out = []
            for inst in bb.instructions:
                si = inst.sync_info
                waits = list(si.on_wait) if si and si.on_wait else []
                if len(waits) > 1:
                    for i, w in enumerate(waits[:-1]):
                        nop = mybir.InstNoOp(
                            name=f"{inst.name}-w{i}",
                            engine=inst.engine,
                            ins=[],
                            outs=[],
                            sync_info=bass_rust.SyncInfo(
                                on_wait=[w], on_update=[]),
                        )
                        out.append(nop)
                        n_new += 1
                    si.on_wait = waits[-1:]
                out.append(inst)
            bb.instructions = out
    return n_new


def _build(samp_mask: np.ndarray, t_run: int = T, split_waits: bool = True):
    """Build the Bass module (specialized on samp_mask). Returns nc."""
    import concourse.bass as bass
    import concourse.tile as tile
    from concourse import mybir

    _patch_drain_wait_limit()

    f32 = mybir.dt.float32
    bf16 = mybir.dt.bfloat16
    AF = mybir.ActivationFunctionType
    ALU = mybir.AluOpType

    mask = [bool(v) for v in samp_mask]
    n_blocks = t_run // OUT_BLK

    nc = bass.Bass()

    # ---- DRAM parameters (host supplies preprocessed layouts) ----
    x_d = nc.declare_dram_parameter("x", [BL, T, D], f32, isOutput=False)
    tp_d = nc.declare_dram_parameter("tp", [BL, T], f32, isOutput=False)
    war_d = nc.declare_dram_parameter("w_ar", [128, 4, 2048], bf16, isOutput=False)
    wtf_d = nc.declare_dram_parameter("w_tf", [128, 4, 1536], bf16, isOutput=False)
    w4ar_d = nc.declare_dram_parameter("w4_ar", [2, 2048], bf16, isOutput=False)
    w4tf_d = nc.declare_dram_parameter("w4_tf", [2, 2048], bf16, isOutput=False)
    wx_d = nc.declare_dram_parameter("w_x", [64, 4, 384], bf16, isOutput=False)
    wout_d = nc.declare_dram_parameter("w_out_k", [128, 4, D], bf16, isOutput=False)
    xbf_d = nc.declare_dram_parameter("x_bf", [BL, T, D], bf16, isOutput=False)
    bout_d = nc.declare_dram_parameter("b_out_c", [D, 1], f32, isOutput=False)
    identb_d = nc.declare_dram_parameter("ident_bf", [128, 128], bf16, isOutput=False)
    mask_d = nc.declare_dram_parameter("mask_f", [1, T], f32, isOutput=False)
    out_d = nc.declare_dram_parameter("out", [BL * T, D], f32, isOutput=True)
    if DEBUG_DUMP:
        dbg_apsum = nc.declare_dram_parameter("dbg_apsum", [128, 512], f32, isOutput=True)
        dbg_hbl = nc.declare_dram_parameter("dbg_hbl", [112, 128], f32, isOutput=True)
        dbg_ring = nc.declare_dram_parameter("dbg_ring", [128, RING * 128], f32, isOutput=True)

    with tile.TileContext(nc) as tc:
        with (
            tc.tile_pool(name="const", bufs=1) as consts,
            tc.tile_pool(name="apsum", bufs=2, space="PSUM") as apsum_pool,
            tc.tile_pool(name="tpsum", bufs=1, space="PSUM") as tpsum_pool,
            tc.tile_pool(name="opsum", bufs=1, space="PSUM") as opsum_pool,
            tc.tile_pool(name="warm", bufs=1, space="PSUM") as warm_pool,
            tc.tile_pool(name="chain", bufs=3) as chain,
            tc.tile_pool(name="hbl", bufs=3) as hbl_pool,
            tc.tile_pool(name="xin", bufs=9) as xin_pool,
            tc.tile_pool(name="osb", bufs=2) as osb_pool,
        ):
            # ---- constants into SBUF ----
            w_ar = consts.tile([128, 4, 2048], bf16)
            nc.sync.dma_start(out=w_ar[:], in_=war_d[:])
            w_tf = consts.tile([128, 4, 1536], bf16)
            nc.sync.dma_start(out=w_tf[:], in_=wtf_d[:])
            w4_ar = consts.tile([2, 2048], bf16)
            nc.sync.dma_start(out=w4_ar[:], in_=w4ar_d[:])
            w4_tf = consts.tile([2, 2048], bf16)
            nc.sync.dma_start(out=w4_tf[:], in_=w4tf_d[:])
            w_x = consts.tile([64, 4, 384], bf16)
            nc.sync.dma_start(out=w_x[:], in_=wx_d[:])
            w_out_k = consts.tile([128, 4, D], bf16)
            nc.sync.dma_start(out=w_out_k[:], in_=wout_d[:])
            b_out_c = consts.tile([D, 1], f32)
            nc.sync.dma_start(out=b_out_c[:], in_=bout_d[:])
            ident_bf = consts.tile([128, 128], bf16)
            nc.sync.dma_start(out=ident_bf[:], in_=identb_d[:])

            # delta_stat row0 holds the per-step "last input channel" in
            # t-major (t,b) order: delta_t for TF steps, x[b,t,63] for AR
            # steps (the reference's ar_in uses x_t[:, -2:-1] = channel 63).
            # row1 = ones (bias injection row).
            delta_stat = consts.tile([2, T * 32], bf16)
            tp_bm = consts.tile([BL, T], f32)
            nc.sync.dma_start(out=tp_bm[:], in_=tp_d[:])
            dl_bm = consts.tile([BL, T], f32)
            nc.vector.tensor_copy(out=dl_bm[:, 0:1], in_=tp_bm[:, 0:1])
            nc.vector.tensor_sub(dl_bm[:, 1:T], tp_bm[:, 1:T], tp_bm[:, 0:T - 1])
            # x63_bm[b, t] = x[b, t, 63]
            x63_bm = consts.tile([BL, T], f32)
            nc.sync.dma_start(
                out=x63_bm[:],
                in_=_apd(x_d[:], D - 1, [[T * D, BL], [D, T]]),
            )
            mask_bm = consts.tile([BL, T], f32)
            nc.sync.dma_start(
                out=mask_bm[:],
                in_=_apd(mask_d[:], 0, [[0, BL], [1, T]]),
            )
            aux_bm = consts.tile([BL, T], f32)
            nc.vector.tensor_sub(aux_bm[:], dl_bm[:], x63_bm[:])
            nc.vector.tensor_mul(aux_bm[:], aux_bm[:], mask_bm[:])
            nc.vector.tensor_add(aux_bm[:], aux_bm[:], x63_bm[:])
            aux_bf = consts.tile([BL, T], bf16)
            nc.vector.tensor_copy(out=aux_bf[:], in_=aux_bm[:])
            nc.vector.memset(delta_stat[:, :], 1.0)  # row1 stays all-ones
            # scatter (BL, T) b-major -> row0 (1, T*BL) t-major; one DMA per row
            for b in range(BL):
                src = aux_bf[b:b + 1, :]
                dst = _ap(delta_stat[0:1, :], b, [[32, T]])
                nc.sync.dma_start(out=dst, in_=src)

            # h-history ring: slot t % RING holds h_arr[t] (= h_state(t+1))
            # slot layout: 4 k-groups x 32 cols; cols 16-31 of each group are
            # zero so 32-col stationaries write full 32-row PSUM strips
            h_ring = consts.tile([128, RING * 128], bf16)
            nc.vector.memset(h_ring[:], 0.0)
            zeros_h = consts.tile([128, 128], bf16)
            nc.vector.memset(zeros_h[:], 0.0)
            h0_bl = consts.tile([112, 128], f32)
            nc.vector.memset(h0_bl[:], 0.0)

            STRIPS = (0, 32, 64, 96)
            hv = h_ring[:].rearrange("p (t g) -> p t g", g=128)

            def emit_out_mms(blk):
                """PE part: project h_arr steps [blk*32, blk*32+32) -> opsum.
                Emitted early so the matmuls fill the gate-phase PE idle."""
                t0 = blk * OUT_BLK
                r0 = t0 % RING
                opsum = opsum_pool.tile([D, OUT_BLK * BL], f32)
                for kb in range(4):
                    rhs = hv[:, r0:r0 + OUT_BLK, 32 * kb:32 * kb + 16]
                    nc.tensor.matmul(
                        opsum[:],
                        w_out_k[:, kb, :],
                        rhs,
                        start=(kb == 0),
                        stop=(kb == 3),
                    )
                return opsum

            def emit_out_tail(blk, opsum):
                """bias add (ScalarE, per-partition bias) + store DMAs.
                Emitted after the gate chain so it never sits ahead of the
                critical-path ops in an engine FIFO."""
                t0 = blk * OUT_BLK
                o_sb = osb_pool.tile([D, OUT_BLK * BL], f32)
                # bias-add writes b-major ((b,t) order) via a strided out AP,
                # so every store DMA below is contiguous on both sides —
                # element-gather DMAs took 4-8us each and clogged the rings
                nc.scalar.activation(_ap(o_sb[:], 0, [[1, OUT_BLK], [OUT_BLK, BL]]),
                                     opsum[:], AF.Identity,
                                     bias=b_out_c[:, 0:1])
                for b in range(BL):
                    dst = _apd(out_d[:], (b * T + t0) * D,
                               [[1, D], [D, OUT_BLK]])
                    nc.sync.dma_start(
                        out=dst,
                        in_=_ap(o_sb[:], b * OUT_BLK, [[1, OUT_BLK]]),
                    )

            xin_tiles = {}
            XAHEAD = 6

            def emit_xin_dma(s):
                """Prefetch the teacher-forcing x_t tile well ahead of use so
                a slow DMA ring never stalls the recurrence."""
                if s < t_run and mask[s]:
                    # explicit per-slot tags force real rotation: the
                    # scheduler otherwise binds same-tag allocs to one slot,
                    # making the prefetch depth-1
                    xt = xin_pool.tile([64, 32], bf16, tag=f"xin{s % 9}",
                                       bufs=1)
                    nc.gpsimd.memset(xt[:, BL:32], 0.0)
                    nc.sync.dma_start(
                        out=xt[:, 0:BL],
                        in_=_apd(xbf_d[:], s * D, [[1, D], [T * D, BL]]),
                    )
                    xin_tiles[s] = xt

            def emit_start_group(t, apsA, apsB):
                """h-independent matmuls opening step t's PSUM accumulation:
                K4 (delta+bias rows, start=True) into both half-tiles and for
                TF steps the x_t @ W_ih contribution."""
                tf = mask[t]
                w4 = w4_tf if tf else w4_ar
                for q, sp in enumerate(STRIPS):
                    nc.tensor.matmul(
                        apsA[sp:sp + 32, 0:256],
                        delta_stat[:, t * 32:(t + 1) * 32],
                        w4[:, 512 * q:512 * q + 256],
                        start=True, stop=False,
                        tile_position=(0, sp),
                        skip_group_check=True,
                    )
                for q, sp in enumerate(STRIPS):
                    nc.tensor.matmul(
                        apsB[sp:sp + 32, 0:256],
                        delta_stat[:, t * 32:(t + 1) * 32],
                        w4[:, 512 * q + 256:512 * q + 512],
                        start=True, stop=False,
                        tile_position=(0, sp),
                        skip_group_check=True,
                    )
                if tf:
                    xin = xin_tiles.pop(t)
                    for q, sp in enumerate(STRIPS):
                        # strip block of w_x is [r | zb | n] (128 each):
                        # r -> A cols 0:128, zb+n -> B cols 0:256
                        nc.tensor.matmul(
                            apsA[sp:sp + 32, 0:128],
                            xin[:],
                            w_x[:, q, 0:128],
                            start=False, stop=False,
                            tile_position=(0, sp),
                            skip_group_check=True,
                        )
                        nc.tensor.matmul(
                            apsB[sp:sp + 32, 0:256],
                            xin[:],
                            w_x[:, q, 128:384],
                            start=False, stop=False,
                            tile_position=(0, sp),
                            skip_group_check=True,
                        )

            h_bl_prev = h0_bl

            # HAM warmup: ~24 back-to-back N=512 matmuls (~10us cold) push the
            # PE activity monitor to K=8/8 before the recurrence starts.
            warm_psum = warm_pool.tile([128, 512], f32, tag="warm")
            for _ in range(24):
                nc.tensor.matmul(
                    warm_psum[0:32, 0:512], zeros_h[:, 0:32], w_ar[:, 0, 0:512],
                    start=True, stop=True, skip_group_check=True,
                )

            def emit_dummies(n):
                """Warm-keeper matmuls into the write-only warm tile: keep the
                PE busy through the gate phase so HAM never re-throttles."""
                for _ in range(n):
                    nc.tensor.matmul(
                        warm_psum[0:32, 0:256], zeros_h[:, 0:32],
                        w_ar[:, 0, 0:256],
                        start=True, stop=True, skip_group_check=True,
                    )

            for s in range(XAHEAD + 1):
                emit_xin_dma(s)

            apsA_cur = apsum_pool.tile([128, 256], f32, tag="apsA0",
                                       bufs=1, padded_shape=[128, 512])
            apsB_cur = apsum_pool.tile([128, 256], f32, tag="apsB0",
                                       bufs=1, padded_shape=[128, 512])
            emit_start_group(0, apsA_cur, apsB_cur)

            for t in range(t_run):
                tf = mask[t]
                apsA, apsB = apsA_cur, apsB_cur

                if t > 0:
                    slot = (t - 1) % RING
                    h_stat = h_ring[:, slot * 128:(slot + 1) * 128]
                else:
                    h_stat = zeros_h[:]

                # ---------- recurrent matmuls ----------
                # strip blocks are [r | hn | zb | in] (AR, 512 wide) or
                # [r | hn | zb] (TF, 384 wide). Subpass A = r,hn -> apsA;
                # subpass B = the rest -> apsB.
                wmov, sw = (w_tf, 384) if tf else (w_ar, 512)
                bw = sw - 256          # B-subpass width (128 TF / 256 AR)
                for k in range(4):
                    lhsT = h_stat[:, 32 * k:32 * k + 32]
                    for q, sp in enumerate(STRIPS):
                        nc.tensor.matmul(
                            apsA[sp:sp + 32, 0:256],
                            lhsT,
                            wmov[:, k, sw * q:sw * q + 256],
                            start=False, stop=(k == 3),
                            tile_position=(0, sp),
                            skip_group_check=True,
                        )
                for k in range(4):
                    lhsT = h_stat[:, 32 * k:32 * k + 32]
                    for q, sp in enumerate(STRIPS):
                        nc.tensor.matmul(
                            apsB[sp:sp + 32, 0:bw],
                            lhsT,
                            wmov[:, k, sw * q + 256:sw * q + sw],
                            start=False, stop=(k == 3),
                            tile_position=(0, sp),
                            skip_group_check=True,
                        )

                emit_xin_dma(t + XAHEAD + 1)

                # ---------- next step's h-independent group ----------
                if t + 1 < t_run:
                    pr = (t + 1) % 2
                    apsA_cur = apsum_pool.tile([128, 256], f32,
                                               tag=f"apsA{pr}", bufs=1,
                                               padded_shape=[128, 512])
                    apsB_cur = apsum_pool.tile([128, 256], f32,
                                               tag=f"apsB{pr}", bufs=1,
                                               padded_shape=[128, 512])
                    emit_start_group(t + 1, apsA_cur, apsB_cur)

                # ---------- interleaved output projection (PE part) ----------
                opsum_pending = None
                if t % OUT_BLK == 0 and t >= OUT_BLK:
                    opsum_pending = (t // OUT_BLK - 1,
                                     emit_out_mms(t // OUT_BLK - 1))

                emit_dummies(6)

                # ---------- gate chain ----------
                # r = sig(rpre); zb = sig(-zpre); z = 1-zb
                # n = tanh(in + r*hn); h' = zb*n + z*h = m1 + m2
                r_sb = chain.tile([112, 128], f32, tag="r")
                nc.scalar.activation(r_sb[:], apsA[0:112, 0:128], AF.Sigmoid)
                zb_sb = chain.tile([112, 128], f32, tag="zb")
                nc.scalar.activation(zb_sb[:], apsB[0:112, 0:128], AF.Sigmoid)
                pre1 = chain.tile([112, 128], f32, tag="pre1")
                nc.vector.tensor_mul(pre1[:], r_sb[:], apsA[0:112, 128:256])
                pre2 = chain.tile([112, 128], f32, tag="pre2")
                nc.vector.tensor_add(pre2[:], pre1[:], apsB[0:112, 128:256])
                n_sb = chain.tile([112, 128], f32, tag="n")
                nc.scalar.activation(n_sb[:], pre2[:], AF.Tanh)
                # z, m2 and the batch-layout h' copy live on GpSimd: its FIFO
                # order is theirs alone, so they can't block the DVE chain
                z_sb = chain.tile([112, 128], f32, tag="z")
                nc.gpsimd.tensor_scalar(z_sb[:], zb_sb[:], -1.0, 1.0,
                                        ALU.mult, ALU.add)
                m2_bf = chain.tile([112, 128], bf16, tag="m2")
                nc.gpsimd.tensor_mul(m2_bf[:], z_sb[:], h_bl_prev[:])
                m1_bf = chain.tile([112, 128], bf16, tag="m1")
                nc.vector.tensor_mul(m1_bf[:], zb_sb[:], n_sb[:])
                hp_bf = chain.tile([112, 128], bf16, tag="hp")
                nc.vector.tensor_add(hp_bf[:], m1_bf[:], m2_bf[:])
                h_bl = hbl_pool.tile([112, 128], f32, tag="hbl")
                nc.gpsimd.tensor_add(h_bl[:], m1_bf[:], m2_bf[:])

                # h'^T on the PE (bf16 transpose), then cast into the ring
                tpsum = tpsum_pool.tile([128, 112], bf16, tag="tp")
                nc.tensor.matmul(tpsum[:], hp_bf[:], ident_bf[0:112, 0:112],
                                 is_transpose=True, start=True, stop=True)
                emit_dummies(4)
                slot = t % RING
                hsrc = _ap(tpsum[:], 0, [[32, 4], [1, 16]])
                hdst = _ap(h_ring[0:128, slot * 128:(slot + 1) * 128], 0,
                           [[32, 4], [1, 16]])
                nc.vector.tensor_copy(out=hdst, in_=hsrc)

                h_bl_prev = h_bl

                if opsum_pending is not None:
                    emit_out_tail(*opsum_pending)

                if DEBUG_DUMP and t == 0:
                    dbg_sb = chain.tile([128, 512], f32, tag="dbgc2")
                    nc.vector.tensor_copy(out=dbg_sb[:, 0:256], in_=apsA[:])
                    nc.vector.tensor_copy(out=dbg_sb[:, 256:512], in_=apsB[:])
                    nc.sync.dma_start(out=dbg_apsum[:], in_=dbg_sb[:])
                    nc.sync.dma_start(out=dbg_hbl[:], in_=h_bl[:])

            if n_blocks > 0:
                emit_out_tail(n_blocks - 1, emit_out_mms(n_blocks - 1))
            if DEBUG_DUMP:
                nc.sync.dma_start(out=dbg_ring[:], in_=h_ring[:])

    if split_waits:
        _split_multi_waits(nc)
    return nc


def _preprocess(W_ih, W_hh, b_ih, b_hh, W_out, b_out):
    """Host-side weight folding into the layouts the kernel expects.

    Strip block order is [r | hn | zb | in] (zb = negated z columns)."""
    f = np.float32
    W_ih = np.asarray(W_ih, f); W_hh = np.asarray(W_hh, f)
    b_ih = np.asarray(b_ih, f); b_hh = np.asarray(b_hh, f)
    W_out = np.asarray(W_out, f); b_out = np.asarray(b_out, f)

    Wih_d = W_ih[:, :D]          # (3H, D)
    w_last = W_ih[:, D]          # (3H,)
    W_gi = W_out @ Wih_d.T       # (H, 3H)
    c_ar = b_out @ Wih_d.T + b_ih

    sl = {"r": slice(0, H), "z": slice(H, 2 * H), "n": slice(2 * H, 3 * H)}

    def strip_pack(cols, width):
        """list of (rows, H) blocks -> (rows, 4*width): per strip q the
        blocks' H-chunk q slices are laid out contiguously."""
        rows = cols[0].shape[0]
        out = np.zeros((rows, 4 * width), f)
        for q in range(4):
            for i, Mfull in enumerate(cols):
                out[:, width * q + 128 * i:width * q + 128 * (i + 1)] = \
                    Mfull[:, 128 * q:128 * q + 128]
        return out

    zero_h = np.zeros((1, H), f)

    # AR moving: per strip [r | hn | zb | in]
    A = [W_hh.T[:, sl["r"]] + W_gi[:, sl["r"]],
         W_hh.T[:, sl["n"]],
         -(W_hh.T[:, sl["z"]] + W_gi[:, sl["z"]]),
         W_gi[:, sl["n"]]]
    w_ar = strip_pack(A, 512)                       # (512, 2048)
    w_ar = w_ar.reshape(4, 128, 2048).transpose(1, 0, 2)

    # TF moving: per strip [r | hn | zb]
    Tm = [W_hh.T[:, sl["r"]], W_hh.T[:, sl["n"]], -W_hh.T[:, sl["z"]]]
    w_tf = strip_pack(Tm, 384)                      # (512, 1536)
    w_tf = w_tf.reshape(4, 128, 1536).transpose(1, 0, 2)

    # K4 rows: row0 = delta coefs, row1 = bias coefs (per strip [r|hn|zb|in])
    w4_ar = np.concatenate([
        strip_pack([w_last[None, sl["r"]], zero_h,
                    -w_last[None, sl["z"]], w_last[None, sl["n"]]], 512),
        strip_pack([(b_hh[sl["r"]] + c_ar[sl["r"]])[None],
                    b_hh[None, sl["n"]],
                    -(b_hh[sl["z"]] + c_ar[sl["z"]])[None],
                    c_ar[None, sl["n"]]], 512),
    ], axis=0)
    w4_tf = np.concatenate([
        strip_pack([w_last[None, sl["r"]], zero_h,
                    -w_last[None, sl["z"]], w_last[None, sl["n"]]], 512),
        strip_pack([(b_ih[sl["r"]] + b_hh[sl["r"]])[None],
                    b_hh[None, sl["n"]],
                    -(b_ih[sl["z"]] + b_hh[sl["z"]])[None],
                    b_ih[None, sl["n"]]], 512),
    ], axis=0)

    # x-side for TF: per strip [r | zb | n] (r -> 0:128, zb+n -> 256:512)
    w_x = strip_pack([Wih_d.T[:, sl["r"]], -Wih_d.T[:, sl["z"]],
                      Wih_d.T[:, sl["n"]]], 384)    # (64, 1536)
    w_x = w_x.reshape(D, 4, 384)

    w_out_k = W_out.reshape(4, 128, D).transpose(1, 0, 2)
    b_out_c = b_out.reshape(D, 1)
    ident = np.eye(128, dtype=f)

    return dict(
        w_ar=np.ascontiguousarray(w_ar), w_tf=np.ascontiguousarray(w_tf),
        w4_ar=np.ascontiguousarray(w4_ar), w4_tf=np.ascontiguousarray(w4_tf),
        w_x=np.ascontiguousarray(w_x),
        w_out_k=np.ascontiguousarray(w_out_k),
        b_out_c=np.ascontiguousarray(b_out_c), ident_bf=ident,
    )


def kernel(x, tp, samp_mask, W_ih, W_hh, b_ih, b_hh, W_out, b_out,
           _trace=False):
    from concourse.bass_utils import run_bass_kernel_spmd

    x = np.ascontiguousarray(np.asarray(x, np.float32))
    tp = np.ascontiguousarray(np.asarray(tp, np.float32))
    samp_mask = np.asarray(samp_mask)

    key = tuple(int(v) for v in samp_mask)
    if key not in _CACHE:
        _CACHE[key] = _build(samp_mask)
    nc = _CACHE[key]

    import ml_dtypes
    bf = ml_dtypes.bfloat16
    wdict = _preprocess(W_ih, W_hh, b_ih, b_hh, W_out, b_out)
    for k in ("w_ar", "w_tf", "w4_ar", "w4_tf", "w_x", "w_out_k", "ident_bf"):
        wdict[k] = np.ascontiguousarray(wdict[k].astype(bf))
    wdict["mask_f"] = np.ascontiguousarray(
        samp_mask.astype(np.float32).reshape(1, T))

    in_maps = []
    for c in range(NC):
        m = dict(wdict)
        m["x"] = np.ascontiguousarray(x[c * BL:(c + 1) * BL])
        m["x_bf"] = np.ascontiguousarray(x[c * BL:(c + 1) * BL].astype(bf))
        m["tp"] = np.ascontiguousarray(tp[c * BL:(c + 1) * BL])
        in_maps.append(m)

    res = run_bass_kernel_spmd(nc, in_maps, list(range(NC)), trace=_trace)
    out = np.concatenate([res.results[c]["out"] for c in range(NC)], axis=0)
    if _trace:
        kernel.last_results = res
    return out.astype(np.float32)


# revision 29
# speedup vs baseline: 1.2064x; 1.2064x over previous
"""Trainium2 Bass kernel for nn_EncoderGRU (B=128, T=512, D=64, H=512).

Strategy
--------
Pure data-parallel over batch: 8 cores x 16 batch rows each. The T=512 GRU
recurrence is inherently sequential, so per-step critical-path latency is
everything.

Per step the PE computes gate pre-activations with h as the stationary
operand and weights moving, with 4-way column tiling (strip q = PSUM
partitions 32q..32q+15 carries the 512-wide gate block for H-chunk q).
Strip layout is [r | hn | zb | in] so the matmul splits into subpass A
(cols 0:256 = r,hn) and subpass B (cols 256:512): sigmoid(r) fires as soon
as A's four k-groups finish, overlapping with B's streaming.

Gate math uses zbar = sigmoid(-z_pre) (z negated host-side) and
z = 1 - zbar, so

    h' = zbar*n + z*h  =  m1 + m2

and the final add is executed ON THE PE as two accumulating bf16
transposes T(m2)+T(m1) into PSUM - fusing the h'-sum with the
batch->H-partition transpose the next step's stationary needs. The
batch-layout copy h_bl' = m1+m2 runs on GpSimd off the critical path.

Critical chain per step:
    A-passes -> sigmoid(r) -> r*hn -> +in -> tanh -> zbar*n -> T(m1) -> cast

All h-independent matmuls (bias/delta K4 row, teacher-forcing x@W_ih, the
interleaved output projection) run on the PE during the gate phase, keeping
the PE HAM-warm.

samp_mask is read host-side; the kernel is specialized per step at build
time (teacher forcing vs autoregressive; AR folds the prev_out feedback
into the recurrent weights: inp = [h @ W_out + b_out, delta] =>
gi = h @ (W_out @ Wih_d.T) + ...).
"""

import os
import sys
import numpy as np

sys.path.insert(0, "/opt/trn_rl_repo")

DEBUG_DUMP = bool(os.environ.get("KERNEL_DEBUG_DUMP"))

B, T, D, H = 128, 512, 64, 512
NC = 8            # cores
BL = B // NC      # local batch = 16
OUT_BLK = 32      # steps per output-projection block
RING = 64         # h-history ring slots (must be multiple of OUT_BLK)

_CACHE = {}


def _ap(base, offset_add, ap_dims):
    """Clone an AP keeping the partition dim, replacing the free dims (SBUF)."""
    import concourse.bass as bass
    return bass.AP(
        tensor=base.tensor,
        offset=base.offset + offset_add,
        ap=[base.ap[0]] + ap_dims,
    )


def _apd(base, offset_add, ap_dims):
    """Clone a DRAM AP with fully custom dims."""
    import concourse.bass as bass
    return bass.AP(
        tensor=base.tensor,
        offset=base.offset + offset_add,
        ap=ap_dims,
    )


def _patch_drain_wait_limit():
    """The walrus build in this image rejects >1 sync wait on a CTRL/Drain
    instruction ("Too many sync wait commands"). Spread the kernel-tail
    drain's waits across multiple drain instructions."""
    import bass_rust
    import concourse.tile as tile
    from concourse.vector_clock import ScopedClock

    if getattr(tile.TileContext, "_drain_patched", False):
        return
    MAXW = 1

    def _drain_and_barrier(self, tick_clock, wait_clock):
        drain_inst = self.nc.sync.drain()
        wait_clock.add_sem_waits(
            drain_inst.ins, ScopedClock({None: tick_clock.global_clock})
        )
        si = drain_inst.ins.sync_info
        waits = list(si.on_wait or [])
        if len(waits) > MAXW:
            si.on_wait = waits[:MAXW]
            rest = waits[MAXW:]
            for i in range(0, len(rest), MAXW):
                d2 = self.nc.sync.drain()
                chunk = rest[i:i + MAXW]
                si2 = d2.ins.sync_info
                if si2 is None:
                    d2.ins.sync_info = bass_rust.SyncInfo(
                        on_wait=chunk, on_update=[])
                else:
                    si2.on_wait = chunk
        self.nc.all_engine_barrier()
        assert self.sems is not None
        popped = self.nc._tile_sem_poison_stack.pop()
        assert popped is self._sem_poison
        self.nc.clear_and_free_semaphores(list(self.sems.allocated().values()))
        self.nc.all_engine_barrier()

    tile.TileContext._drain_and_barrier = _drain_and_barrier
    tile.TileContext._drain_patched = True


def _split_multi_waits(nc):
    """The walrus build in this image accepts at most ONE sync wait per
    instruction. Hoist extra waits onto preceding single-wait NoOps on the
    same engine (engine sequencers execute in order, so semantics hold)."""
    import bass_rust
    from concourse import mybir

    n_new = 0
    for fn in nc.m.functions:
        for bb in fn.blocks:
            out = []
            for inst in bb.instructions:
                si = inst.sync_info
                waits = list(si.on_wait) if si and si.on_wait else []
                if len(waits) > 1:
                    for i, w in enumerate(waits[:-1]):
                        nop = mybir.InstNoOp(
                            name=f"{inst.name}-w{i}",
                            engine=inst.engine,
                            ins=[],
                            outs=[],
                            sync_info=bass_rust.SyncInfo(
                                on_wait=[w], on_update=[]),
                        )
                        out.append(nop)
                        n_new += 1
                    si.on_wait = waits[-1:]
                out.append(inst)
            bb.instructions = out
    return n_new


def _build(samp_mask: np.ndarray, t_run: int = T, split_waits: bool = True):
    """Build the Bass module (specialized on samp_mask). Returns nc."""
    import concourse.bass as bass
    import concourse.tile as tile
    from concourse import mybir

    _patch_drain_wait_limit()

    f32 = mybir.dt.float32
    bf16 = mybir.dt.bfloat16
    AF = mybir.ActivationFunctionType
    ALU = mybir.AluOpType

    mask = [bool(v) for v in samp_mask]
    n_blocks = t_run // OUT_BLK

    nc = bass.Bass()

    # ---- DRAM parameters (host supplies preprocessed layouts) ----
    x_d = nc.declare_dram_parameter("x", [BL, T, D], f32, isOutput=False)
    tp_d = nc.declare_dram_parameter("tp", [BL, T], f32, isOutput=False)
    war_d = nc.declare_dram_parameter("w_ar", [128, 4, 2048], bf16, isOutput=False)
    wtf_d = nc.declare_dram_parameter("w_tf", [128, 4, 1536], bf16, isOutput=False)
    w4ar_d = nc.declare_dram_parameter("w4_ar", [2, 2048], bf16, isOutput=False)
    w4tf_d = nc.declare_dram_parameter("w4_tf", [2, 2048], bf16, isOutput=False)
    wx_d = nc.declare_dram_parameter("w_x", [64, 4, 384], bf16, isOutput=False)
    wout_d = nc.declare_dram_parameter("w_out_k", [128, 4, D], bf16, isOutput=False)
    xbf_d = nc.declare_dram_parameter("x_bf", [BL, T, D], bf16, isOutput=False)
    bout_d = nc.declare_dram_parameter("b_out_c", [D, 1], f32, isOutput=False)
    identb_d = nc.declare_dram_parameter("ident_bf", [128, 128], bf16, isOutput=False)
    mask_d = nc.declare_dram_parameter("mask_f", [1, T], f32, isOutput=False)
    out_d = nc.declare_dram_parameter("out", [BL * T, D], f32, isOutput=True)
    if DEBUG_DUMP:
        dbg_apsum = nc.declare_dram_parameter("dbg_apsum", [128, 512], f32, isOutput=True)
        dbg_hbl = nc.declare_dram_parameter("dbg_hbl", [112, 128], f32, isOutput=True)
        dbg_ring = nc.declare_dram_parameter("dbg_ring", [128, RING * 128], f32, isOutput=True)

    with tile.TileContext(nc) as tc:
        with (
            tc.tile_pool(name="const", bufs=1) as consts,
            tc.tile_pool(name="apsum", bufs=2, space="PSUM") as apsum_pool,
            tc.tile_pool(name="tpsum", bufs=1, space="PSUM") as tpsum_pool,
            tc.tile_pool(name="opsum", bufs=1, space="PSUM") as opsum_pool,
            tc.tile_pool(name="warm", bufs=1, space="PSUM") as warm_pool,
            tc.tile_pool(name="chain", bufs=3) as chain,
            tc.tile_pool(name="hbl", bufs=3) as hbl_pool,
            tc.tile_pool(name="xin", bufs=9) as xin_pool,
            tc.tile_pool(name="osb", bufs=2) as osb_pool,
        ):
            # ---- constants into SBUF ----
            w_ar = consts.tile([128, 4, 2048], bf16)
            nc.sync.dma_start(out=w_ar[:], in_=war_d[:])
            w_tf = consts.tile([128, 4, 1536], bf16)
            nc.sync.dma_start(out=w_tf[:], in_=wtf_d[:])
            w4_ar = consts.tile([2, 2048], bf16)
            nc.sync.dma_start(out=w4_ar[:], in_=w4ar_d[:])
            w4_tf = consts.tile([2, 2048], bf16)
            nc.sync.dma_start(out=w4_tf[:], in_=w4tf_d[:])
            w_x = consts.tile([64, 4, 384], bf16)
            nc.sync.dma_start(out=w_x[:], in_=wx_d[:])
            w_out_k = consts.tile([128, 4, D], bf16)
            nc.sync.dma_start(out=w_out_k[:], in_=wout_d[:])
            b_out_c = consts.tile([D, 1], f32)
            nc.sync.dma_start(out=b_out_c[:], in_=bout_d[:])
            ident_bf = consts.tile([128, 128], bf16)
            nc.sync.dma_start(out=ident_bf[:], in_=identb_d[:])

            # delta_stat row0 holds the per-step "last input channel" in
            # t-major (t,b) order: delta_t for TF steps, x[b,t,63] for AR
            # steps (the reference's ar_in uses x_t[:, -2:-1] = channel 63).
            # row1 = ones (bias injection row).
            delta_stat = consts.tile([2, T * 32], bf16)
            tp_bm = consts.tile([BL, T], f32)
            nc.sync.dma_start(out=tp_bm[:], in_=tp_d[:])
            dl_bm = consts.tile([BL, T], f32)
            nc.vector.tensor_copy(out=dl_bm[:, 0:1], in_=tp_bm[:, 0:1])
            nc.vector.tensor_sub(dl_bm[:, 1:T], tp_bm[:, 1:T], tp_bm[:, 0:T - 1])
            # x63_bm[b, t] = x[b, t, 63]
            x63_bm = consts.tile([BL, T], f32)
            nc.sync.dma_start(
                out=x63_bm[:],
                in_=_apd(x_d[:], D - 1, [[T * D, BL], [D, T]]),
            )
            mask_bm = consts.tile([BL, T], f32)
            nc.sync.dma_start(
                out=mask_bm[:],
                in_=_apd(mask_d[:], 0, [[0, BL], [1, T]]),
            )
            aux_bm = consts.tile([BL, T], f32)
            nc.vector.tensor_sub(aux_bm[:], dl_bm[:], x63_bm[:])
            nc.vector.tensor_mul(aux_bm[:], aux_bm[:], mask_bm[:])
            nc.vector.tensor_add(aux_bm[:], aux_bm[:], x63_bm[:])
            aux_bf = consts.tile([BL, T], bf16)
            nc.vector.tensor_copy(out=aux_bf[:], in_=aux_bm[:])
            nc.vector.memset(delta_stat[:, :], 1.0)  # row1 stays all-ones
            # scatter (BL, T) b-major -> row0 (1, T*BL) t-major; one DMA per row
            for b in range(BL):
                src = aux_bf[b:b + 1, :]
                dst = _ap(delta_stat[0:1, :], b, [[32, T]])
                nc.sync.dma_start(out=dst, in_=src)

            # h-history ring: slot t % RING holds h_arr[t] (= h_state(t+1))
            # slot layout: 4 k-groups x 32 cols; cols 16-31 of each group are
            # zero so 32-col stationaries write full 32-row PSUM strips
            h_ring = consts.tile([128, RING * 128], bf16)
            nc.vector.memset(h_ring[:], 0.0)
            zeros_h = consts.tile([128, 128], bf16)
            nc.vector.memset(zeros_h[:], 0.0)
            h0_bl = consts.tile([112, 128], f32)
            nc.vector.memset(h0_bl[:], 0.0)

            STRIPS = (0, 32, 64, 96)
            hv = h_ring[:].rearrange("p (t g) -> p t g", g=128)

            def emit_out_mms(blk):
                """PE part: project h_arr steps [blk*32, blk*32+32) -> opsum.
                Emitted early so the matmuls fill the gate-phase PE idle."""
                t0 = blk * OUT_BLK
                r0 = t0 % RING
                opsum = opsum_pool.tile([D, OUT_BLK * BL], f32)
                for kb in range(4):
                    rhs = hv[:, r0:r0 + OUT_BLK, 32 * kb:32 * kb + 16]
                    nc.tensor.matmul(
                        opsum[:],
                        w_out_k[:, kb, :],
                        rhs,
                        start=(kb == 0),
                        stop=(kb == 3),
                    )
                return opsum

            def emit_out_tail(blk, opsum):
                """bias add (ScalarE, per-partition bias) + store DMAs.
                Emitted after the gate chain so it never sits ahead of the
                critical-path ops in an engine FIFO."""
                t0 = blk * OUT_BLK
                o_sb = osb_pool.tile([D, OUT_BLK * BL], f32)
                # bias-add writes b-major ((b,t) order) via a strided out AP,
                # so every store DMA below is contiguous on both sides —
                # element-gather DMAs took 4-8us each and clogged the rings
                nc.scalar.activation(_ap(o_sb[:], 0, [[1, OUT_BLK], [OUT_BLK, BL]]),
                                     opsum[:], AF.Identity,
                                     bias=b_out_c[:, 0:1])
                for b in range(BL):
                    dst = _apd(out_d[:], (b * T + t0) * D,
                               [[1, D], [D, OUT_BLK]])
                    nc.sync.dma_start(
                        out=dst,
                        in_=_ap(o_sb[:], b * OUT_BLK, [[1, OUT_BLK]]),
                    )

            xin_tiles = {}
            XAHEAD = 6

            def emit_xin_dma(s):
                """Prefetch the teacher-forcing x_t tile well ahead of use so
                a slow DMA ring never stalls the recurrence."""
                if s < t_run and mask[s]:
                    # explicit per-slot tags force real rotation: the
                    # scheduler otherwise binds same-tag allocs to one slot,
                    # making the prefetch depth-1
                    xt = xin_pool.tile([64, 32], bf16, tag=f"xin{s % 9}",
                                       bufs=1)
                    nc.gpsimd.memset(xt[:, BL:32], 0.0)
                    nc.sync.dma_start(
                        out=xt[:, 0:BL],
                        in_=_apd(xbf_d[:], s * D, [[1, D], [T * D, BL]]),
                    )
                    xin_tiles[s] = xt

            def emit_start_group(t, apsA, apsB):
                """h-independent matmuls opening step t's PSUM accumulation:
                K4 (delta+bias rows, start=True) into both half-tiles and for
                TF steps the x_t @ W_ih contribution."""
                tf = mask[t]
                w4 = w4_tf if tf else w4_ar
                for q, sp in enumerate(STRIPS):
                    nc.tensor.matmul(
                        apsA[sp:sp + 32, 0:256],
                        delta_stat[:, t * 32:(t + 1) * 32],
                        w4[:, 512 * q:512 * q + 256],
                        start=True, stop=False,
                        tile_position=(0, sp),
                        skip_group_check=True,
                    )
                for q, sp in enumerate(STRIPS):
                    nc.tensor.matmul(
                        apsB[sp:sp + 32, 0:256],
                        delta_stat[:, t * 32:(t + 1) * 32],
                        w4[:, 512 * q + 256:512 * q + 512],
                        start=True, stop=False,
                        tile_position=(0, sp),
                        skip_group_check=True,
                    )
                if tf:
                    xin = xin_tiles.pop(t)
                    for q, sp in enumerate(STRIPS):
                        # strip block of w_x is [r | zb | n] (128 each):
                        # r -> A cols 0:128, zb+n -> B cols 0:256
                        nc.tensor.matmul(
                            apsA[sp:sp + 32, 0:128],
                            xin[:],
                            w_x[:, q, 0:128],
                            start=False, stop=False,
                            tile_position=(0, sp),
                            skip_group_check=True,
                        )
                        nc.tensor.matmul(
                            apsB[sp:sp + 32, 0:256],
                            xin[:],
                            w_x[:, q, 128:384],
                            start=False, stop=False,
                            tile_position=(0, sp),
                            skip_group_check=True,
                        )

            h_bl_prev = h0_bl

            # HAM warmup: ~24 back-to-back N=512 matmuls (~10us cold) push the
            # PE activity monitor to K=8/8 before the recurrence starts.
            warm_psum = warm_pool.tile([128, 512], f32, tag="warm")
            for _ in range(24):
                nc.tensor.matmul(
                    warm_psum[0:32, 0:512], zeros_h[:, 0:32], w_ar[:, 0, 0:512],
                    start=True, stop=True, skip_group_check=True,
                )

            def emit_dummies(n):
                """Warm-keeper matmuls into the write-only warm tile: keep the
                PE busy through the gate phase so HAM never re-throttles."""
                for _ in range(n):
                    nc.tensor.matmul(
                        warm_psum[0:32, 0:256], zeros_h[:, 0:32],
                        w_ar[:, 0, 0:256],
                        start=True, stop=True, skip_group_check=True,
                    )

            for s in range(XAHEAD + 1):
                emit_xin_dma(s)

            apsA_cur = apsum_pool.tile([128, 256], f32, tag="apsA0",
                                       bufs=1, padded_shape=[128, 512])
            apsB_cur = apsum_pool.tile([128, 256], f32, tag="apsB0",
                                       bufs=1, padded_shape=[128, 512])
            emit_start_group(0, apsA_cur, apsB_cur)

            for t in range(t_run):
                tf = mask[t]
                apsA, apsB = apsA_cur, apsB_cur

                if t > 0:
                    slot = (t - 1) % RING
                    h_stat = h_ring[:, slot * 128:(slot + 1) * 128]
                else:
                    h_stat = zeros_h[:]

                # ---------- recurrent matmuls ----------
                # strip blocks are [r | hn | zb | in] (AR, 512 wide) or
                # [r | hn | zb] (TF, 384 wide). Subpass A = r,hn -> apsA;
                # subpass B = the rest -> apsB.
                wmov, sw = (w_tf, 384) if tf else (w_ar, 512)
                bw = sw - 256          # B-subpass width (128 TF / 256 AR)
                for k in range(4):
                    lhsT = h_stat[:, 32 * k:32 * k + 32]
                    for q, sp in enumerate(STRIPS):
                        nc.tensor.matmul(
                            apsA[sp:sp + 32, 0:256],
                            lhsT,
                            wmov[:, k, sw * q:sw * q + 256],
                            start=False, stop=(k == 3),
                            tile_position=(0, sp),
                            skip_group_check=True,
                        )
                for k in range(4):
                    lhsT = h_stat[:, 32 * k:32 * k + 32]
                    for q, sp in enumerate(STRIPS):
                        nc.tensor.matmul(
                            apsB[sp:sp + 32, 0:bw],
                            lhsT,
                            wmov[:, k, sw * q + 256:sw * q + sw],
                            start=False, stop=(k == 3),
                            tile_position=(0, sp),
                            skip_group_check=True,
                        )

                emit_xin_dma(t + XAHEAD + 1)

                # ---------- next step's h-independent group ----------
                if t + 1 < t_run:
                    pr = (t + 1) % 2
                    apsA_cur = apsum_pool.tile([128, 256], f32,
                                               tag=f"apsA{pr}", bufs=1,
                                               padded_shape=[128, 512])
                    apsB_cur = apsum_pool.tile([128, 256], f32,
                                               tag=f"apsB{pr}", bufs=1,
                                               padded_shape=[128, 512])
                    emit_start_group(t + 1, apsA_cur, apsB_cur)

                # ---------- interleaved output projection (PE part) ----------
                opsum_pending = None
                if t % OUT_BLK == 0 and t >= OUT_BLK:
                    opsum_pending = (t // OUT_BLK - 1,
                                     emit_out_mms(t // OUT_BLK - 1))

                emit_dummies(6)

                # ---------- gate chain ----------
                # r = sig(rpre); zb = sig(-zpre); z = 1-zb
                # n = tanh(in + r*hn); h' = zb*n + z*h = m1 + m2
                r_sb = chain.tile([112, 128], f32, tag="r")
                nc.scalar.activation(r_sb[:], apsA[0:112, 0:128], AF.Sigmoid)
                zb_sb = chain.tile([112, 128], f32, tag="zb")
                nc.scalar.activation(zb_sb[:], apsB[0:112, 0:128], AF.Sigmoid)
                pre1 = chain.tile([112, 128], f32, tag="pre1")
                nc.vector.tensor_mul(pre1[:], r_sb[:], apsA[0:112, 128:256])
                pre2 = chain.tile([112, 128], f32, tag="pre2")
                nc.vector.tensor_add(pre2[:], pre1[:], apsB[0:112, 128:256])
                n_sb = chain.tile([112, 128], f32, tag="n")
                nc.scalar.activation(n_sb[:], pre2[:], AF.Tanh)
                # z, m2 and the batch-layout h' copy live on GpSimd: its FIFO
                # order is theirs alone, so they can't block the DVE chain
                z_sb = chain.tile([112, 128], f32, tag="z")
                nc.gpsimd.tensor_scalar(z_sb[:], zb_sb[:], -1.0, 1.0,
                                        ALU.mult, ALU.add)
                m2_bf = chain.tile([112, 128], bf16, tag="m2")
                nc.gpsimd.tensor_mul(m2_bf[:], z_sb[:], h_bl_prev[:])
                # T(m2) transposes early (during tanh); T(m1) right after m1.
                # The ring write then fuses the h' = m1+m2 add with the cast:
                # ring = bf16(T(m2)); ring += T(m1)  (each TT has 1 PSUM src)
                tp2 = tpsum_pool.tile([128, 112], bf16, tag="tp2")
                nc.tensor.matmul(tp2[:], m2_bf[:], ident_bf[0:112, 0:112],
                                 is_transpose=True, start=True, stop=True)
                m1_bf = chain.tile([112, 128], bf16, tag="m1")
                nc.vector.tensor_mul(m1_bf[:], zb_sb[:], n_sb[:])
                h_bl = hbl_pool.tile([112, 128], f32, tag="hbl")
                nc.gpsimd.tensor_add(h_bl[:], m1_bf[:], m2_bf[:])
                tp1 = tpsum_pool.tile([128, 112], bf16, tag="tp1")
                nc.tensor.matmul(tp1[:], m1_bf[:], ident_bf[0:112, 0:112],
                                 is_transpose=True, start=True, stop=True)
                emit_dummies(6)
                slot = t % RING
                ring_w = h_ring[0:128, slot * 128:(slot + 1) * 128]
                hdst = _ap(ring_w, 0, [[32, 4], [1, 16]])
                nc.vector.tensor_copy(out=hdst,
                                      in_=_ap(tp2[:], 0, [[32, 4], [1, 16]]))
                nc.vector.tensor_add(_ap(ring_w, 0, [[32, 4], [1, 16]]),
                                     _ap(ring_w, 0, [[32, 4], [1, 16]]),
                                     _ap(tp1[:], 0, [[32, 4], [1, 16]]))

                h_bl_prev = h_bl

                if opsum_pending is not None:
                    emit_out_tail(*opsum_pending)

                if DEBUG_DUMP and t == 0:
                    dbg_sb = chain.tile([128, 512], f32, tag="dbgc2")
                    nc.vector.tensor_copy(out=dbg_sb[:, 0:256], in_=apsA[:])
                    nc.vector.tensor_copy(out=dbg_sb[:, 256:512], in_=apsB[:])
                    nc.sync.dma_start(out=dbg_apsum[:], in_=dbg_sb[:])
                    nc.sync.dma_start(out=dbg_hbl[:], in_=h_bl[:])

            if n_blocks > 0:
                emit_out_tail(n_blocks - 1, emit_out_mms(n_blocks - 1))
            if DEBUG_DUMP:
                nc.sync.dma_start(out=dbg_ring[:], in_=h_ring[:])

    if split_waits:
        _split_multi_waits(nc)
    return nc


def _preprocess(W_ih, W_hh, b_ih, b_hh, W_out, b_out):
    """Host-side weight folding into the layouts the kernel expects.

    Strip block order is [r | hn | zb | in] (zb = negated z columns)."""
    f = np.float32
    W_ih = np.asarray(W_ih, f); W_hh = np.asarray(W_hh, f)
    b_ih = np.asarray(b_ih, f); b_hh = np.asarray(b_hh, f)
    W_out = np.asarray(W_out, f); b_out = np.asarray(b_out, f)

    Wih_d = W_ih[:, :D]          # (3H, D)
    w_last = W_ih[:, D]          # (3H,)
    W_gi = W_out @ Wih_d.T       # (H, 3H)
    c_ar = b_out @ Wih_d.T + b_ih

    sl = {"r": slice(0, H), "z": slice(H, 2 * H), "n": slice(2 * H, 3 * H)}

    def strip_pack(cols, width):
        """list of (rows, H) blocks -> (rows, 4*width): per strip q the
        blocks' H-chunk q slices are laid out contiguously."""
        rows = cols[0].shape[0]
        out = np.zeros((rows, 4 * width), f)
        for q in range(4):
            for i, Mfull in enumerate(cols):
                out[:, width * q + 128 * i:width * q + 128 * (i + 1)] = \
                    Mfull[:, 128 * q:128 * q + 128]
        return out

    zero_h = np.zeros((1, H), f)

    # AR moving: per strip [r | hn | zb | in]
    A = [W_hh.T[:, sl["r"]] + W_gi[:, sl["r"]],
         W_hh.T[:, sl["n"]],
         -(W_hh.T[:, sl["z"]] + W_gi[:, sl["z"]]),
         W_gi[:, sl["n"]]]
    w_ar = strip_pack(A, 512)                       # (512, 2048)
    w_ar = w_ar.reshape(4, 128, 2048).transpose(1, 0, 2)

    # TF moving: per strip [r | hn | zb]
    Tm = [W_hh.T[:, sl["r"]], W_hh.T[:, sl["n"]], -W_hh.T[:, sl["z"]]]
    w_tf = strip_pack(Tm, 384)                      # (512, 1536)
    w_tf = w_tf.reshape(4, 128, 1536).transpose(1, 0, 2)

    # K4 rows: row0 = delta coefs, row1 = bias coefs (per strip [r|hn|zb|in])
    w4_ar = np.concatenate([
        strip_pack([w_last[None, sl["r"]], zero_h,
                    -w_last[None, sl["z"]], w_last[None, sl["n"]]], 512),
        strip_pack([(b_hh[sl["r"]] + c_ar[sl["r"]])[None],
                    b_hh[None, sl["n"]],
                    -(b_hh[sl["z"]] + c_ar[sl["z"]])[None],
                    c_ar[None, sl["n"]]], 512),
    ], axis=0)
    w4_tf = np.concatenate([
        strip_pack([w_last[None, sl["r"]], zero_h,
                    -w_last[None, sl["z"]], w_last[None, sl["n"]]], 512),
        strip_pack([(b_ih[sl["r"]] + b_hh[sl["r"]])[None],
                    b_hh[None, sl["n"]],
                    -(b_ih[sl["z"]] + b_hh[sl["z"]])[None],
                    b_ih[None, sl["n"]]], 512),
    ], axis=0)

    # x-side for TF: per strip [r | zb | n] (r -> 0:128, zb+n -> 256:512)
    w_x = strip_pack([Wih_d.T[:, sl["r"]], -Wih_d.T[:, sl["z"]],
                      Wih_d.T[:, sl["n"]]], 384)    # (64, 1536)
    w_x = w_x.reshape(D, 4, 384)

    w_out_k = W_out.reshape(4, 128, D).transpose(1, 0, 2)
    b_out_c = b_out.reshape(D, 1)
    ident = np.eye(128, dtype=f)

    return dict(
        w_ar=np.ascontiguousarray(w_ar), w_tf=np.ascontiguousarray(w_tf),
        w4_ar=np.ascontiguousarray(w4_ar), w4_tf=np.ascontiguousarray(w4_tf),
        w_x=np.ascontiguousarray(w_x),
        w_out_k=np.ascontiguousarray(w_out_k),
        b_out_c=np.ascontiguousarray(b_out_c), ident_bf=ident,
    )


def kernel(x, tp, samp_mask, W_ih, W_hh, b_ih, b_hh, W_out, b_out,
           _trace=False):
    from concourse.bass_utils import run_bass_kernel_spmd

    x = np.ascontiguousarray(np.asarray(x, np.float32))
    tp = np.ascontiguousarray(np.asarray(tp, np.float32))
    samp_mask = np.asarray(samp_mask)

    key = tuple(int(v) for v in samp_mask)
    if key not in _CACHE:
        _CACHE[key] = _build(samp_mask)
    nc = _CACHE[key]

    import ml_dtypes
    bf = ml_dtypes.bfloat16
    wdict = _preprocess(W_ih, W_hh, b_ih, b_hh, W_out, b_out)
    for k in ("w_ar", "w_tf", "w4_ar", "w4_tf", "w_x", "w_out_k", "ident_bf"):
        wdict[k] = np.ascontiguousarray(wdict[k].astype(bf))
    wdict["mask_f"] = np.ascontiguousarray(
        samp_mask.astype(np.float32).reshape(1, T))

    in_maps = []
    for c in range(NC):
        m = dict(wdict)
        m["x"] = np.ascontiguousarray(x[c * BL:(c + 1) * BL])
        m["x_bf"] = np.ascontiguousarray(x[c * BL:(c + 1) * BL].astype(bf))
        m["tp"] = np.ascontiguousarray(tp[c * BL:(c + 1) * BL])
        in_maps.append(m)

    res = run_bass_kernel_spmd(nc, in_maps, list(range(NC)), trace=_trace)
    out = np.concatenate([res.results[c]["out"] for c in range(NC)], axis=0)
    if _trace:
        kernel.last_results = res
    return out.astype(np.float32)
